# revision 23
# baseline (speedup 1.0000x reference)
"""Trainium2 Bass kernel for nn_Classifier (segment mean-pool + tiny MLP head).

Pipeline (matches the jax reference):
  pooled[g] = mean of features over nodes with batch id g   (2048 graphs)
  out = LeakyReLU(LayerNorm(pooled @ W1 + b1)) @ W2 + b2    -> [2048, 1]

Sharding strategy: the batch ids are sorted, so nodes are split across the 8
cores at segment-block boundaries — core i owns graphs [256i, 256i+256) and
exactly the nodes belonging to them. Each core computes segment sums for its
own 256 graphs (disjoint), so no collective is needed; the host concatenates
the 8 per-core [256]-sized outputs.

Per core, segment sums are computed on the tensor engine: for each 128-node
sub-tile, a one-hot matrix [128 nodes, 128 segs] is used as the matmul
stationary against the node features [128, 256], accumulating into PSUM.

v2 changes vs the 561us baseline (bottleneck was one-hot generation via
tensor_scalar with a per-partition scalar operand — the TensorScalarPtr
slow path, 1236ns/op on DVE and 2180ns/op on Pool, 94% Pool busy):
  * one-hots for a whole 8-sub-tile chunk are built by a single wide
    tensor_tensor(is_equal) on the DVE: iota [128, 8x128] vs the chunk's
    segment ids broadcast along the seg axis (stride-0 AP). ~1.2us per
    1024 nodes instead of ~13us.
  * features (and the one-hot/iota/segids) stream as fp16 — halves HBM
    traffic, and the PE runs 16-bit matmuls at 1 cycle/row. Segment sums
    still accumulate in fp32 PSUM; only input rounding (2^-11) is lost.
  * per-segment reciprocal counts are computed on the host from `batch`
    (the host already scans it for shard boundaries) and shipped as a
    [128, 2] input — drops the appended ones column from the feature
    stream and the on-device count math.
  * feature-chunk DMAs round-robin over three queue engines (sync,
    scalar, pool).
  * the MLP head is evaluated per 128-segment region as soon as that
    region's sums finish, so region 0's head overlaps region 1's stream.
"""

from contextlib import ExitStack

import numpy as np

import concourse.bass as bass
import concourse.mybir as mybir
import concourse.tile as tile
from concourse.bass_utils import run_bass_kernel_spmd

# ---------------------------------------------------------------------------
# Workaround: this walrus build rejects instructions carrying more than one
# semaphore wait ("Too many sync wait commands"), but Tile's semaphore
# assignment freely attaches several. After the TileContext has lowered the
# program, split any excess waits onto same-engine nops inserted right before
# the instruction (semantics are identical: all waits are monotonic and must
# hold before the instruction issues).
_MAX_WAITS = 1


def _split_excess_waits(nc: "bass.Bass", max_waits: int = _MAX_WAITS) -> None:
    ctr = 0
    for f in nc.m.functions:
        for b in f.blocks:
            out = []
            for inst in b.instructions:
                si = inst.sync_info
                waits = list(si.on_wait) if (si is not None and si.on_wait) else []
                if len(waits) > max_waits:
                    keep = waits[-max_waits:]
                    extra = waits[:-max_waits]
                    # On the PE queue the carrier must be a DRAIN: silicon
                    # promotes waitless LDWEIGHTS past in-flight work, so a
                    # plain nop's wait can be bypassed (walrus attaches a
                    # matmul's waits to its LDWEIGHTS — stripping them onto a
                    # nop re-opens that race). A drain fully serializes.
                    is_pe = inst.engine == mybir.EngineType.PE
                    for i in range(0, len(extra), max_waits):
                        ctr += 1
                        if is_pe:
                            nop = mybir.InstDrain(
                                name=f"waitsplit_drain_{ctr}", ins=[], outs=[],
                                engine=inst.engine,
                            )
                        else:
                            nop = mybir.InstNoOp(
                                name=f"waitsplit_nop_{ctr}", ins=[], outs=[],
                                engine=inst.engine,
                            )
                        nop.sync_info = mybir.SyncInfo(
                            on_wait=extra[i : i + max_waits], on_update=[]
                        )
                        nc.register_instruction(nop)
                        out.append(nop)
                    inst.sync_info = mybir.SyncInfo(
                        on_wait=keep, on_update=list(si.on_update or [])
                    )
                out.append(inst)
            b.instructions = out
# ---------------------------------------------------------------------------

N_CORES = 8
NUM_GRAPHS = 2048
SEGS_PER_CORE = NUM_GRAPHS // N_CORES  # 256
D = 256
K_SUB = 8  # 128-node sub-tiles per DMA chunk (chunk = 1024 nodes, 512 KB fp16)
CHUNK = 128 * K_SUB
LN_EPS = 1e-5
NEG_SLOPE = 0.01

_F32 = mybir.dt.float32
_F16 = mybir.dt.float16
_ALU = mybir.AluOpType

# Test/debug hooks: set PROFILE=True before calling kernel() to request an
# NTFF trace; the BassKernelResults lands in LAST_RESULT.
PROFILE = False
PROFILE_DIR = None
LAST_RESULT = None


def _build_program(chunks_per_region: int) -> bass.Bass:
    R = chunks_per_region
    C = 2 * R  # chunks per core (2 segment blocks of 128)
    n_nodes = C * CHUNK

    nc = bass.Bass("TRN2", debug=False)
    feat = nc.dram_tensor("feat", [n_nodes, D], _F16, kind="ExternalInput").ap()
    segT = nc.dram_tensor("segT", [128, C * K_SUB], _F16, kind="ExternalInput").ap()
    ident_d = nc.dram_tensor("ident", [128, 128], _F32, kind="ExternalInput").ap()
    w1aug_d = nc.dram_tensor("w1aug", [D + 1, 128], _F32, kind="ExternalInput").ap()
    pvec_d = nc.dram_tensor("pvec", [1, 385], _F32, kind="ExternalInput").ap()
    rcnt_d = nc.dram_tensor("rcnt", [128, 2], _F32, kind="ExternalInput").ap()
    out_d = nc.dram_tensor("out", [1, 256], _F32, kind="ExternalOutput").ap()

    with tile.TileContext(nc) as tc, ExitStack() as ctx:
        cpool = ctx.enter_context(tc.tile_pool(name="consts", bufs=1))
        fpool = ctx.enter_context(tc.tile_pool(name="feat", bufs=12))
        opool = ctx.enter_context(tc.tile_pool(name="oh", bufs=6))
        acc = ctx.enter_context(tc.tile_pool(name="acc", bufs=1, space="PSUM"))
        ppool = ctx.enter_context(tc.tile_pool(name="pw", bufs=1, space="PSUM"))
        spool = ctx.enter_context(tc.tile_pool(name="small", bufs=2))

        # the one-hot compare needs segT (DMA, gpsimd queue) and iota
        # (generated on the DVE — 128 tiny DMA lines would be slower);
        # the feature chunks then lead the sync/scalar queues.
        segT_t = cpool.tile([128, C * K_SUB], _F16, tag="segT")
        nc.gpsimd.dma_start(out=segT_t[:], in_=segT[:])
        iota_t = cpool.tile([128, 128], _F16, tag="iota")
        nc.gpsimd.iota(
            iota_t[:], pattern=[[1, 128]], base=0, channel_multiplier=0,
            allow_small_or_imprecise_dtypes=True,
        )
        # head-only consts are DMA'd mid-stream (see the chunk loop below) so
        # the three DMA queues start on feature chunks immediately; tiles are
        # just allocated here.
        ident_t = cpool.tile([128, 128], _F32, tag="ident")
        w1a = cpool.tile([128, 128], _F32, tag="w1a")
        w1b = cpool.tile([128, 128], _F32, tag="w1b")
        w1c = cpool.tile([1, 128], _F32, tag="w1c")
        pv = cpool.tile([1, 385], _F32, tag="pv")
        rcnt_t = cpool.tile([128, 2], _F32, tag="rcnt")
        ones_row = cpool.tile([1, 128], _F32, tag="ones")
        out_sb = cpool.tile([1, 256], _F32, tag="outsb")
        ones_col = cpool.tile([128, 1], _F32, tag="onesc")
        epsc = cpool.tile([128, 1], _F32, tag="epsc")
        bc = cpool.tile([128, 385], _F32, tag="bcs")

        def _late_consts():
            # two small DMAs per queue, slotted behind each queue's first
            # feature chunk
            nc.sync.dma_start(out=ident_t[:], in_=ident_d[:])
            nc.sync.dma_start(out=w1a[:], in_=w1aug_d[0:128, :])
            nc.scalar.dma_start(out=w1b[:], in_=w1aug_d[128:256, :])
            nc.scalar.dma_start(out=w1c[:], in_=w1aug_d[256:257, :])
            nc.gpsimd.dma_start(out=pv[:], in_=pvec_d[:])
            nc.gpsimd.dma_start(out=rcnt_t[:], in_=rcnt_d[:])
            nc.vector.memset(ones_row[:], 1.0)
            nc.vector.memset(ones_col[:], 1.0)
            nc.vector.memset(epsc[:], LN_EPS)

        # ---- main stream: per-segment sums, one 128-seg region at a time ----
        sums = [acc.tile([128, D], _F32, tag=f"sum{r}", name=f"sum{r}") for r in range(2)]
        dma_engs = (nc.sync, nc.scalar, nc.gpsimd)
        for r in range(2):
            for c in range(R):
                chunk = r * R + c
                if chunk == 3:
                    _late_consts()
                ft = fpool.tile([128, K_SUB, D], _F16, tag="ft")
                src = feat[chunk * CHUNK : (chunk + 1) * CHUNK, :].rearrange(
                    "(p k) f -> p k f", p=128
                )
                dma_engs[chunk % 3].dma_start(out=ft[:], in_=src)
                # all 8 one-hots of the chunk in one wide DVE compare:
                # oh[p, k, s] = (iota[s] == segid[p, chunk*8+k])
                oh = opool.tile([128, K_SUB, 128], _F16, tag="oh")
                in0 = iota_t[:].unsqueeze(1).broadcast_to([128, K_SUB, 128])
                in1 = (
                    segT_t[:, chunk * K_SUB : (chunk + 1) * K_SUB]
                    .unsqueeze(2)
                    .broadcast_to([128, K_SUB, 128])
                )
                nc.vector.tensor_tensor(out=oh[:], in0=in0, in1=in1, op=_ALU.is_equal)
                for k in range(K_SUB):
                    nc.tensor.matmul(
                        out=sums[r][:],
                        lhsT=oh[:, k, :],
                        rhs=ft[:, k, :],
                        start=(c == 0 and k == 0),
                        stop=(c == R - 1 and k == K_SUB - 1),
                    )

            # ---- region tail: pooled mean, transpose, MLP head for this
            # region's 128 graphs (overlaps the next region's stream) ----
            if r == 0:
                # broadcast [gamma | beta | W2 | b2] to all 128 partitions
                bc_ps = ppool.tile([128, 385], _F32, tag="bc")
                nc.tensor.matmul(
                    out=bc_ps[:], lhsT=ones_row[:], rhs=pv[:], start=True, stop=True
                )
                nc.scalar.copy(bc[:], bc_ps[:])
            pooled = spool.tile([128, D], _F32, tag="pooled")
            nc.vector.scalar_tensor_tensor(
                out=pooled[:], in0=sums[r][:], scalar=rcnt_t[:, r : r + 1],
                in1=ones_col[:].broadcast_to([128, D]),
                op0=_ALU.mult, op1=_ALU.mult,
            )
            ptT = []
            for fb in range(2):
                tp = ppool.tile([128, 128], _F32, tag="tp")
                nc.tensor.transpose(
                    out=tp[:], in_=pooled[:, fb * 128 : (fb + 1) * 128],
                    identity=ident_t[:],
                )
                pt = spool.tile([128, 128], _F32, tag=f"ptT{fb}")
                nc.scalar.copy(pt[:], tp[:])
                ptT.append(pt)

            # h = pooled @ W1 + b1; LayerNorm; LeakyReLU; @ W2 + b2
            h_ps = ppool.tile([128, 128], _F32, tag="h")
            nc.tensor.matmul(
                out=h_ps[:], lhsT=ptT[0][:], rhs=w1a[:], start=True, stop=False
            )
            nc.tensor.matmul(
                out=h_ps[:], lhsT=ptT[1][:], rhs=w1b[:], start=False, stop=False
            )
            nc.tensor.matmul(
                out=h_ps[:], lhsT=ones_row[:], rhs=w1c[:], start=False, stop=True
            )

            stats = spool.tile([128, 6], _F32, tag="stats")
            nc.vector.bn_stats(stats[:], h_ps[:])
            aggr = spool.tile([128, 2], _F32, tag="aggr")
            nc.vector.bn_aggr(aggr[:], stats[:])
            std = spool.tile([128, 1], _F32, tag="std")
            nc.scalar.activation(
                std[:], aggr[:, 1:2], mybir.ActivationFunctionType.Sqrt,
                bias=epsc[:], scale=1.0,
            )
            rstd = spool.tile([128, 1], _F32, tag="rstd")
            nc.vector.reciprocal(rstd[:], std[:])
            hc = spool.tile([128, 128], _F32, tag="hc")
            nc.vector.tensor_scalar(
                out=hc[:], in0=h_ps[:], scalar1=aggr[:, 0:1], scalar2=None,
                op0=_ALU.subtract,
            )
            y = spool.tile([128, 128], _F32, tag="y")
            nc.vector.scalar_tensor_tensor(
                out=y[:], in0=hc[:], scalar=rstd[:], in1=bc[:, 0:128],
                op0=_ALU.mult, op1=_ALU.mult,
            )
            y2 = spool.tile([128, 128], _F32, tag="y2")
            nc.vector.tensor_tensor(out=y2[:], in0=y[:], in1=bc[:, 128:256],
                                    op=_ALU.add)
            yl = spool.tile([128, 128], _F32, tag="yl")
            nc.vector.scalar_tensor_tensor(
                out=yl[:], in0=y2[:], scalar=NEG_SLOPE, in1=y2[:],
                op0=_ALU.mult, op1=_ALU.max,
            )
            prod = spool.tile([128, 128], _F32, tag="prod")
            oc = spool.tile([128, 1], _F32, tag="oc")
            nc.vector.scalar_tensor_tensor(
                out=prod[:], in0=yl[:], scalar=1.0, in1=bc[:, 256:384],
                op0=_ALU.mult, op1=_ALU.mult, accum_out=oc[:],
            )
            # transpose [128,1] -> a contiguous [1,128] row so the final DMA
            # is one 512B line instead of 128 scattered 4B descriptors; the
            # b2 bias is added by a second accumulating matmul (bc's b2
            # column sums against ident's single 1 per output position)
            ot_ps = ppool.tile([1, 128], _F32, tag="ot")
            nc.tensor.matmul(
                out=ot_ps[:], lhsT=oc[:], rhs=ident_t[:], start=True, stop=False
            )
            nc.tensor.matmul(
                out=ot_ps[:], lhsT=bc[:, 384:385], rhs=ident_t[:], start=False,
                stop=True,
            )
            nc.scalar.copy(out_sb[:, r * 128 : (r + 1) * 128], ot_ps[:])
        nc.sync.dma_start(out=out_d[:], in_=out_sb[:])

    _split_excess_waits(nc)
    return nc


def _prep_inputs(features, batch):
    """Segment-block-aligned sharding + per-core padded fp16 arrays."""
    feats = np.asarray(features)
    seg = np.asarray(batch).astype(np.int64)
    counts = np.bincount(seg, minlength=NUM_GRAPHS)
    bnd = np.zeros(NUM_GRAPHS + 1, np.int64)
    bnd[1:] = np.cumsum(counts)

    block_lo = bnd[0 : NUM_GRAPHS : 128]
    block_hi = bnd[128 : NUM_GRAPHS + 1 : 128]
    block_n = block_hi - block_lo  # nodes per 128-segment block (16 blocks)
    R = int(np.max((block_n + CHUNK - 1) // CHUNK))  # chunks per region
    region = R * CHUNK
    ncap = 2 * region

    feat16 = np.zeros((N_CORES, ncap, D), np.float16)
    seg_adj = np.full((N_CORES, ncap), -1.0, np.float16)
    for i in range(N_CORES):
        for r in range(2):
            b = 2 * i + r
            lo, hi = int(block_lo[b]), int(block_hi[b])
            m = hi - lo
            off = r * region
            feat16[i, off : off + m, :] = feats[lo:hi]
            seg_adj[i, off : off + m] = (seg[lo:hi] - 128 * b).astype(np.float16)
    # transpose seg ids to match the on-chip [partition, sub-tile] layout:
    # node (chunk*1024 + p*8 + k) -> segT[p, chunk*8 + k]
    segT = (
        seg_adj.reshape(N_CORES, -1, 128, K_SUB)
        .transpose(0, 2, 1, 3)
        .reshape(N_CORES, 128, -1)
    )
    # reciprocal counts: rcnt[i, p, r] = 1 / max(count[graph 128*(2i+r)+p], 1)
    rc = (1.0 / np.maximum(counts, 1)).astype(np.float32).reshape(16, 128)
    rcnt = np.stack(
        [np.stack([rc[2 * i], rc[2 * i + 1]], axis=1) for i in range(N_CORES)]
    )
    return feat16, np.ascontiguousarray(segT), rcnt, R


def kernel(features, batch, W1, b1, gamma, beta, W2, b2):
    feat16, segT, rcnt, R = _prep_inputs(features, batch)

    ident = np.eye(128, dtype=np.float32)
    w1aug = np.concatenate(
        [np.asarray(W1, np.float32), np.asarray(b1, np.float32)[None, :]], axis=0
    )
    pvec = np.concatenate(
        [
            np.asarray(gamma, np.float32).ravel(),
            np.asarray(beta, np.float32).ravel(),
            np.asarray(W2, np.float32).ravel(),
            np.asarray(b2, np.float32).ravel(),
        ]
    )[None, :]

    nc = _build_program(R)
    in_maps = [
        {
            "feat": feat16[i],
            "segT": segT[i],
            "ident": ident,
            "w1aug": w1aug,
            "pvec": pvec,
            "rcnt": rcnt[i],
        }
        for i in range(N_CORES)
    ]
    res = run_bass_kernel_spmd(
        nc, in_maps, list(range(N_CORES)), trace=PROFILE, tmpdir=PROFILE_DIR
    )
    global LAST_RESULT
    LAST_RESULT = res
    out = np.concatenate(
        [res.results[i]["out"].reshape(SEGS_PER_CORE) for i in range(N_CORES)]
    )
    return out.reshape(NUM_GRAPHS, 1).astype(np.float32)
